# revision 1
# baseline (speedup 1.0000x reference)
"""YOLO loss (nms_detection) Trainium2 Bass kernel.

Data parallel over 8 NeuronCores (4 images per core). Per (image, layer):
  - y_true is host-augmented with per-cell (gx, gy, aw, ah) -> 89 channels,
    so one TensorEngine gather fetches labels + grid + anchors together.
  - inputs are host-cast to fp16 and cells quad-packed per partition row
    so every DMA descriptor moves >= 680B contiguous at half the bytes.
  - decode pred boxes (sigmoid via exp+reciprocal; one ACT table set).
  - obj compaction: row cumsum (tensor_tensor_scan) + triangular-matmul
    partition offsets -> rank; one-hot S = (iota == rank*obj).
  - gather true-box rows via fp16 matmuls (256-wide two-block rhs);
    dense decode reads a small fp32 copy of the conf/xy/wh channels.
  - broadcast box quantities via DRAM-roundtrip DMA.
  - IoU ignore mask in fp16: big [128, S, M] broadcast-AP DVE ops testing
    3*inter >= a1+a2 (equiv. IoU >= 0.5, no division).
  - dense conf BCE on c = sigmoid(x) with weight max(obj, ignore)*valid;
    obj-masked xy/wh/cls losses on the gathered [M, 174] rows only.
"""

from contextlib import ExitStack

import numpy as np

ANCHORS = np.array([[116., 90.], [156., 198.], [373., 326.],
                    [30., 61.], [62., 45.], [59., 119.],
                    [10., 13.], [16., 30.], [33., 23.]], dtype=np.float32)
IMG_W = 416.0
P = 128
B_CORE = 4
N_CORES = 8
YW = 89           # augmented y_true row: 85 + (gx, gy, aw, ah)
PW = 85
RW = 4 * YW + 4 * PW   # 696: [ytA..ytD | predA..predD] (fp16, quad cells)
TAILPAD = 48      # gather rhs reads up to row_base + 696 + 43 -> pad 48
SW = 174          # gathered sparse row: yt_aug 89 + pred 85
PADV = -60.0

# per-layer: N cells, slots S (=ceil(N/128) padded even), grid W, offsets
LAYERS = [
    dict(N=507,  S=4,  W=13.0, coff=0,    goff=0),
    dict(N=2028, S=16, W=26.0, coff=507,  goff=4),
    dict(N=8112, S=64, W=52.0, coff=2535, goff=20),
]
STOT = 84

_NC_CACHE = {}


def _make_consts():
    # dense grid/anchor const: (gxw, gyw, awhalf, ahhalf, valid)
    gad = np.zeros((P, STOT, 5), np.float32)
    # per-cell ga columns appended to y_true, in flat cell order
    percell = np.zeros((10647, 4), np.float32)
    for li, lay in enumerate(LAYERS):
        W = int(lay["W"])
        N, S, goff, coff = lay["N"], lay["S"], lay["goff"], lay["coff"]
        c = np.arange(N)
        percell[coff:coff + N, 0] = (c % (W * 3)) // 3
        percell[coff:coff + N, 1] = c // (W * 3)
        percell[coff:coff + N, 2] = ANCHORS[3 * li + (c % 3), 0]
        percell[coff:coff + N, 3] = ANCHORS[3 * li + (c % 3), 1]
        p = np.arange(P)[:, None]
        s = np.arange(S)[None, :]
        cell = (s // 4) * 512 + 4 * p + (s % 4)
        valid = cell < N
        cc = np.minimum(cell, N - 1)
        aw = ANCHORS[3 * li + (cc % 3), 0]
        ah = ANCHORS[3 * li + (cc % 3), 1]
        gx = ((cc % (W * 3)) // 3).astype(np.float32)
        gy = (cc // (W * 3)).astype(np.float32)
        gad[:, goff:goff + S, 0] = np.where(valid, gx / W, 0)
        gad[:, goff:goff + S, 1] = np.where(valid, gy / W, 0)
        gad[:, goff:goff + S, 2] = np.where(valid, aw / (2.0 * W), 0)
        gad[:, goff:goff + S, 3] = np.where(valid, ah / (2.0 * W), 0)
        gad[:, goff:goff + S, 4] = valid.astype(np.float32)
    ut = np.triu(np.ones((P, P), np.float32), 1)  # ut[q,p]=1 iff q<p
    ones128 = np.ones((P, 1), np.float32)
    sel = np.zeros((P, B_CORE), np.float32)
    for i in range(B_CORE):
        sel[32 * i:32 * (i + 1), i] = 1.0
    return {"gad": gad, "ut": ut, "ones128": ones128, "sel": sel}, percell


def build_nc(Ms):
    import concourse.bass as bass
    import concourse.bacc as bacc
    import concourse.mybir as mybir
    from concourse.tile import TileContext

    F32 = mybir.dt.float32
    F16 = mybir.dt.float16
    F32R = mybir.dt.float32r
    ALU = mybir.AluOpType
    ACT = mybir.ActivationFunctionType
    AX = mybir.AxisListType
    MM = max(Ms)

    nc = bacc.Bacc()
    yt_d = nc.dram_tensor("yt", [B_CORE, 10647, YW], F16,
                          kind="ExternalInput")
    pr_d = [nc.dram_tensor(f"p{i}", [B_CORE, LAYERS[i]["N"], PW], F16,
                           kind="ExternalInput") for i in range(3)]
    pf_d = nc.dram_tensor("pf", [B_CORE, 10647, 5], F32,
                          kind="ExternalInput")
    ga_d = nc.dram_tensor("gad", [P, STOT, 5], F32, kind="ExternalInput")
    ut_d = nc.dram_tensor("ut", [P, P], F32, kind="ExternalInput")
    on_d = nc.dram_tensor("ones128", [P, 1], F32, kind="ExternalInput")
    se_d = nc.dram_tensor("sel", [P, B_CORE], F32, kind="ExternalInput")
    loss_d = nc.dram_tensor("loss", [B_CORE, 1], F32, kind="ExternalOutput")

    def bmid(ap2, n):
        # [P, X] -> [P, n, X] (step-0 middle dim)
        return bass.AP(tensor=ap2.tensor, offset=ap2.offset,
                       ap=[ap2.ap[0], [0, n]] + ap2.ap[1:])

    big = MM > 32   # fallback config must fit SBUF with M=64
    with TileContext(nc) as tc, ExitStack() as ctx:
        cpool = ctx.enter_context(tc.tile_pool(name="consts", bufs=1))
        combp = {li: ctx.enter_context(
            tc.tile_pool(name=f"comb{li}",
                         bufs=1 if (big and li == 2) else 3))
                 for li in range(3)}
        decp = ctx.enter_context(tc.tile_pool(name="dec", bufs=2 if big else 4))
        ioup = ctx.enter_context(tc.tile_pool(name="iou", bufs=1 if big else 2))
        stp = ctx.enter_context(tc.tile_pool(name="st", bufs=2 if big else 4))
        gatp = ctx.enter_context(
            tc.tile_pool(name="gat", bufs=2 if big else 3))
        spap = ctx.enter_context(tc.tile_pool(name="spa", bufs=1))
        accp = ctx.enter_context(tc.tile_pool(name="acc", bufs=1))
        drp = ctx.enter_context(
            tc.tile_pool(name="scr", bufs=3, space=bass.MemorySpace.DRAM))
        psg = ctx.enter_context(
            tc.tile_pool(name="psg", bufs=3, space=bass.MemorySpace.PSUM))
        pso = ctx.enter_context(
            tc.tile_pool(name="pso", bufs=2, space=bass.MemorySpace.PSUM))

        GAD = cpool.tile([P, STOT, 5], F32)
        nc.sync.dma_start(out=GAD, in_=ga_d[:])
        UT = cpool.tile([P, P], F32)
        nc.sync.dma_start(out=UT, in_=ut_d[:])
        ON128 = cpool.tile([P, 1], F32)
        nc.sync.dma_start(out=ON128, in_=on_d[:])
        SELC = cpool.tile([P, B_CORE], F32)
        nc.sync.dma_start(out=SELC, in_=se_d[:])
        IOTA = cpool.tile([P, MM], F32)
        nc.gpsimd.iota(IOTA[:], [[1, MM]], base=1, channel_multiplier=0,
                       allow_small_or_imprecise_dtypes=True)
        ZER = cpool.tile([P, 64], F32)
        nc.gpsimd.memset(ZER[:], 0.0)

        ACCD = accp.tile([P, B_CORE * 9], F32)   # (img, layer, term) dense
        SACC = accp.tile([P, 9], F32)            # (layer, term) sparse
        nc.gpsimd.memset(SACC[:], 0.0)
        SPA = {li: spap.tile([P, SW], F32, tag=f"spa{li}", name=f"spa{li}")
               for li in range(3)}
        for li in range(3):
            nc.gpsimd.memset(SPA[li][:], 0.0)

        for img, li in [(i, l) for l in (2, 1, 0) for i in range(B_CORE)]:
            if True:
                lay = LAYERS[li]
                N, S, W, coff, goff = (lay["N"], lay["S"], lay["W"],
                                       lay["coff"], lay["goff"])
                M = Ms[li]
                Gp = S // 4                # quad rows
                full = N // 512            # full quad rows
                remc = N - full * 512
                rem_p = remc // 4
                odd = remc % 4             # 0..3 extra cells on one partition
                CF = combp[li].tile([P, Gp * RW + TAILPAD], F16,
                                    tag=f"comb{li}", name=f"comb{li}_{img}")
                cfl = CF[:]
                pstride = cfl.ap[0]

                def yv(c0, c1, _a=cfl, _g=Gp):
                    # yt view [P, Gp, 4, c1-c0]
                    return bass.AP(tensor=_a.tensor, offset=_a.offset + c0,
                                   ap=[_a.ap[0], [RW, _g], [YW, 4],
                                       [1, c1 - c0]])

                def pv(c0, c1, _a=cfl, _g=Gp):
                    return bass.AP(tensor=_a.tensor,
                                   offset=_a.offset + 4 * YW + c0,
                                   ap=[_a.ap[0], [RW, _g], [PW, 4],
                                       [1, c1 - c0]])

                def cview(off, n, _a=cfl):
                    return bass.AP(tensor=_a.tensor, offset=_a.offset + off,
                                   ap=[_a.ap[0], [1, n]])

                # pad init: tail cols + last quad row (dma overwrites live)
                nc.vector.memset(cview(Gp * RW, TAILPAD), 0.0)
                if remc:
                    nc.vector.memset(cview((Gp - 1) * RW, 4 * YW), 0.0)
                    nc.vector.memset(cview((Gp - 1) * RW + 4 * YW, 4 * PW),
                                     PADV)
                # ---- loads (contiguous >= 680B elements) ----
                ysrc = yt_d[img]
                ybase = ysrc.offset + coff * YW
                if full:
                    nc.sync.dma_start(
                        out=bass.AP(tensor=cfl.tensor, offset=cfl.offset,
                                    ap=[[pstride[0], P], [RW, full],
                                        [1, 4 * YW]]),
                        in_=bass.AP(tensor=ysrc.tensor, offset=ybase,
                                    ap=[[4 * YW, P], [512 * YW, full],
                                        [1, 4 * YW]]))
                if rem_p:
                    nc.sync.dma_start(
                        out=bass.AP(tensor=cfl.tensor,
                                    offset=cfl.offset + full * RW,
                                    ap=[[pstride[0], rem_p], [1, 4 * YW]]),
                        in_=bass.AP(tensor=ysrc.tensor,
                                    offset=ybase + full * 512 * YW,
                                    ap=[[4 * YW, rem_p], [1, 4 * YW]]))
                if odd:
                    nc.sync.dma_start(
                        out=CF[rem_p:rem_p + 1,
                               full * RW:full * RW + odd * YW],
                        in_=bass.AP(
                            tensor=ysrc.tensor,
                            offset=ybase + (full * 512 + 4 * rem_p) * YW,
                            ap=[[odd * YW, 1], [1, odd * YW]]))
                psrc = pr_d[li][img]
                pbase = psrc.offset
                if full:
                    nc.sync.dma_start(
                        out=bass.AP(tensor=cfl.tensor,
                                    offset=cfl.offset + 4 * YW,
                                    ap=[[pstride[0], P], [RW, full],
                                        [1, 4 * PW]]),
                        in_=bass.AP(tensor=psrc.tensor, offset=pbase,
                                    ap=[[4 * PW, P], [512 * PW, full],
                                        [1, 4 * PW]]))
                if rem_p:
                    nc.sync.dma_start(
                        out=bass.AP(tensor=cfl.tensor,
                                    offset=cfl.offset + full * RW + 4 * YW,
                                    ap=[[pstride[0], rem_p], [1, 4 * PW]]),
                        in_=bass.AP(tensor=psrc.tensor,
                                    offset=pbase + full * 512 * PW,
                                    ap=[[4 * PW, rem_p], [1, 4 * PW]]))
                if odd:
                    nc.sync.dma_start(
                        out=CF[rem_p:rem_p + 1,
                               full * RW + 4 * YW:full * RW + 4 * YW +
                               odd * PW],
                        in_=bass.AP(
                            tensor=psrc.tensor,
                            offset=pbase + (full * 512 + 4 * rem_p) * PW,
                            ap=[[odd * PW, 1], [1, odd * PW]]))

                # fp32 front pred channels (conf, xy, wh) for dense decode
                FW = 20  # 4 cells x 5 ch
                PF = decp.tile([P, Gp * FW + 20], F32, tag="pf",
                               name=f"pf{li}_{img}")
                pfl = PF[:]
                pfsrc = pf_d[img]
                pfbase = pfsrc.offset + coff * 5
                if remc:
                    nc.vector.memset(
                        bass.AP(tensor=pfl.tensor,
                                offset=pfl.offset + (Gp - 1) * FW,
                                ap=[pfl.ap[0], [1, FW]]), PADV)
                nc.vector.memset(
                    bass.AP(tensor=pfl.tensor, offset=pfl.offset + Gp * FW,
                            ap=[pfl.ap[0], [1, 20]]), PADV)
                if full:
                    nc.sync.dma_start(
                        out=bass.AP(tensor=pfl.tensor, offset=pfl.offset,
                                    ap=[[pfl.ap[0][0], P], [FW, full],
                                        [1, FW]]),
                        in_=bass.AP(tensor=pfsrc.tensor, offset=pfbase,
                                    ap=[[FW, P], [512 * 5, full], [1, FW]]))
                if rem_p:
                    nc.sync.dma_start(
                        out=bass.AP(tensor=pfl.tensor,
                                    offset=pfl.offset + full * FW,
                                    ap=[[pfl.ap[0][0], rem_p], [1, FW]]),
                        in_=bass.AP(tensor=pfsrc.tensor,
                                    offset=pfbase + full * 512 * 5,
                                    ap=[[FW, rem_p], [1, FW]]))
                if odd:
                    nc.sync.dma_start(
                        out=PF[rem_p:rem_p + 1,
                               full * FW:full * FW + odd * 5],
                        in_=bass.AP(
                            tensor=pfsrc.tensor,
                            offset=pfbase + (full * 512 + 4 * rem_p) * 5,
                            ap=[[odd * 5, 1], [1, odd * 5]]))

                def pfv(c0, c1, _a=pfl, _g=Gp):
                    return bass.AP(tensor=_a.tensor, offset=_a.offset + c0,
                                   ap=[_a.ap[0], [FW, _g], [5, 4],
                                       [1, c1 - c0]])

                # compact copies of the interleaved dense channels
                OBJC = decp.tile([P, S], F32, tag="objc")
                oc = OBJC[:]
                nc.vector.tensor_copy(
                    bass.AP(tensor=oc.tensor, offset=oc.offset,
                            ap=[oc.ap[0], [4, Gp], [1, 4]]),
                    yv(0, 1).squeeze(3))
                XCF = decp.tile([P, S], F32, tag="xcf")
                xc = XCF[:]
                nc.scalar.copy(
                    bass.AP(tensor=xc.tensor, offset=xc.offset,
                            ap=[xc.ap[0], [4, Gp], [1, 4]]),
                    pfv(0, 1).squeeze(3))

                def compact2(tile):   # [P, Gp, 4, 2] view over [P, S, 2]
                    a = tile[:]
                    return bass.AP(tensor=a.tensor, offset=a.offset,
                                   ap=[a.ap[0], [8, Gp], [2, 4], [1, 2]])

                # ---- decode dense ----
                EXY = decp.tile([P, S, 2], F32, tag="exy")
                nc.scalar.activation(compact2(EXY), pfv(1, 3), ACT.Exp,
                                     scale=-1.0)
                nc.vector.tensor_scalar_add(EXY[:], EXY[:], 1.0)
                SGX = decp.tile([P, S, 2], F32, tag="sgx")
                nc.vector.reciprocal(SGX[:], EXY[:])
                CXY = decp.tile([P, S, 2], F32, tag="cxy")
                nc.vector.scalar_tensor_tensor(
                    CXY[:], SGX[:], 1.0 / W, GAD[:, goff:goff + S, 0:2],
                    ALU.mult, ALU.add)
                EWH = decp.tile([P, S, 2], F32, tag="ewh")
                nc.scalar.activation(compact2(EWH), pfv(3, 5), ACT.Exp)
                HWT = decp.tile([P, S, 2], F32, tag="hwt")
                nc.vector.tensor_mul(HWT[:], EWH[:],
                                     GAD[:, goff:goff + S, 2:4])
                PMX = decp.tile([P, S, 2], F16, tag="pmx")
                nc.vector.tensor_add(PMX[:], CXY[:], HWT[:])
                PMN = decp.tile([P, S, 2], F16, tag="pmn")
                nc.vector.tensor_sub(PMN[:], CXY[:], HWT[:])
                A13 = decp.tile([P, S], F16, tag="a13")
                nc.vector.scalar_tensor_tensor(
                    A13[:], HWT[:, :, 0], 4.0 / 3.0, HWT[:, :, 1],
                    ALU.mult, ALU.mult)

                # ---- rank & one-hot selection ----
                RCUM = decp.tile([P, S], F32, tag="rcum")
                nc.vector.tensor_tensor_scan(RCUM[:], OBJC[:], ZER[:, 0:S],
                                             0.0, ALU.add, ALU.add)
                OFFP = pso.tile([P, 1], F32, tag="offp")
                nc.tensor.matmul(OFFP[:], UT[:], RCUM[:, S - 1:S],
                                 start=True, stop=True)
                RANK = decp.tile([P, S], F32, tag="rank")
                nc.vector.tensor_scalar_add(RANK[:], RCUM[:], OFFP[:])
                RPM = decp.tile([P, S], F32, tag="rpm")
                nc.vector.tensor_mul(RPM[:], RANK[:], OBJC[:])
                STT = stp.tile([P, S, M], F16, tag="st")
                nc.vector.tensor_tensor(STT[:], bmid(IOTA[:, 0:M], S),
                                        RPM[:].broadcast_to([P, S, M]),
                                        ALU.is_equal)

                # ---- gather true rows (PE, fp16, 256-wide 2-block rhs) ----
                PGA = psg.tile([MM, 256], F32, tag="pga")
                for s in range(S):
                    g, j = s // 4, s % 4
                    yoff = g * RW + j * YW
                    delta = 4 * YW + j * PW - j * YW  # 356 - 4*j
                    rhs = bass.AP(tensor=cfl.tensor,
                                  offset=cfl.offset + yoff,
                                  ap=[[pstride[0], P], [delta, 2], [1, 128]])
                    nc.tensor.matmul(PGA[0:M, :], STT[:, s, :],
                                     rhs, start=(s == 0), stop=(s == S - 1))
                SPT = gatp.tile([MM, SW], F32, tag="spt")
                nc.scalar.copy(SPT[0:M, 0:YW], PGA[0:M, 0:YW])
                nc.scalar.copy(SPT[0:M, YW:SW], PGA[0:M, 128:128 + PW])
                nc.sync.dma_start(out=SPA[li][32 * img:32 * img + M, :],
                                  in_=SPT[0:M, :])
                # box rows (obj,x,y,w,h) -> dram -> [P,5,M] broadcast
                SCR = drp.tile([5, MM], F32, tag="scr")
                s1 = SPT[0:M, 0:5]
                s1t = bass.AP(tensor=s1.tensor, offset=s1.offset,
                              ap=[s1.ap[0], [1, 5], [1, 1]])
                d1 = bass.AP(tensor=SCR[:].tensor, offset=SCR[:].offset,
                             ap=[[1, M], [MM, 5], [1, 1]])
                nc.sync.dma_start(out=d1, in_=s1t)
                RAWB = gatp.tile([P, 5, MM], F32, tag="rawb")
                s2 = bass.AP(tensor=SCR[:].tensor, offset=SCR[:].offset,
                             ap=[[0, P], [1, 5 * MM]])
                d2 = bass.AP(tensor=RAWB[:].tensor, offset=RAWB[:].offset,
                             ap=[RAWB[:].ap[0], [1, 5 * MM]])
                nc.sync.dma_start(out=d2, in_=s2)

                BT = gatp.tile([P, 5, MM], F16, tag="bt")
                X_, Y_, W_, H_ = (RAWB[:, 1, 0:M], RAWB[:, 2, 0:M],
                                  RAWB[:, 3, 0:M], RAWB[:, 4, 0:M])
                nc.vector.scalar_tensor_tensor(BT[:, 0, 0:M], W_, -0.5, X_,
                                               ALU.mult, ALU.add)
                nc.vector.scalar_tensor_tensor(BT[:, 1, 0:M], W_, 0.5, X_,
                                               ALU.mult, ALU.add)
                nc.vector.scalar_tensor_tensor(BT[:, 2, 0:M], H_, -0.5, Y_,
                                               ALU.mult, ALU.add)
                nc.vector.scalar_tensor_tensor(BT[:, 3, 0:M], H_, 0.5, Y_,
                                               ALU.mult, ALU.add)
                nc.vector.scalar_tensor_tensor(BT[:, 4, 0:M], W_, 1.0 / 3.0,
                                               H_, ALU.mult, ALU.mult)

                # ---- IoU ignore: smax = max_m(rx*ry - a1/3 - a2/3) ----
                shp = [P, S, M]
                IX = ioup.tile(shp, F16, tag="ix")
                nc.vector.tensor_tensor(IX[:], PMX[:, :, 0].broadcast_to(shp),
                                        bmid(BT[:, 1, 0:M], S), ALU.min)
                JX = ioup.tile(shp, F16, tag="jx")
                nc.vector.tensor_tensor(JX[:], PMN[:, :, 0].broadcast_to(shp),
                                        bmid(BT[:, 0, 0:M], S), ALU.max)
                nc.vector.tensor_sub(IX[:], IX[:], JX[:])
                nc.scalar.activation(IX[:], IX[:], ACT.Relu)
                IY = ioup.tile(shp, F16, tag="iy")
                nc.vector.tensor_tensor(IY[:], PMX[:, :, 1].broadcast_to(shp),
                                        bmid(BT[:, 3, 0:M], S), ALU.min)
                JY = ioup.tile(shp, F16, tag="jy")
                nc.vector.tensor_tensor(JY[:], PMN[:, :, 1].broadcast_to(shp),
                                        bmid(BT[:, 2, 0:M], S), ALU.max)
                nc.vector.tensor_sub(IY[:], IY[:], JY[:])
                nc.scalar.activation(IY[:], IY[:], ACT.Relu)
                nc.vector.tensor_mul(JX[:], IX[:], IY[:])
                nc.vector.tensor_tensor(JX[:], JX[:],
                                        A13[:].broadcast_to(shp),
                                        ALU.subtract)
                nc.vector.tensor_tensor(JX[:], JX[:], bmid(BT[:, 4, 0:M], S),
                                        ALU.subtract)
                SMX = decp.tile([P, S], F32, tag="smx")
                nc.vector.tensor_reduce(SMX[:], JX[:], axis=AX.X, op=ALU.max)

                # ---- dense conf loss (on c = sigmoid(x)) ----
                WT = decp.tile([P, S], F32, tag="wt")
                nc.vector.scalar_tensor_tensor(WT[:], SMX[:], 0.0, OBJC[:],
                                               ALU.is_lt, ALU.max)
                nc.vector.tensor_mul(WT[:], WT[:], GAD[:, goff:goff + S, 4])
                ECF = decp.tile([P, S], F32, tag="ecf")
                nc.scalar.activation(ECF[:], XCF[:], ACT.Exp, scale=-1.0)
                nc.vector.tensor_scalar_add(ECF[:], ECF[:], 1.0)
                CCF = decp.tile([P, S], F32, tag="ccf")
                nc.vector.reciprocal(CCF[:], ECF[:])
                E3T = decp.tile([P, S], F32, tag="e3t")
                nc.scalar.activation(E3T[:], CCF[:], ACT.Exp, scale=-1.0)
                L1T = decp.tile([P, S], F32, tag="l1t")
                nc.scalar.activation(L1T[:], E3T[:], ACT.Ln, bias=1.0)
                SCRP = decp.tile([P, S], F32, tag="scrp")
                base = img * 9 + li * 3
                nc.vector.scalar_tensor_tensor(
                    SCRP[:], CCF[:], 1.0, WT[:], ALU.mult, ALU.mult,
                    accum_out=ACCD[:, base:base + 1])
                nc.vector.scalar_tensor_tensor(
                    SCRP[:], L1T[:], 1.0, WT[:], ALU.mult, ALU.mult,
                    accum_out=ACCD[:, base + 1:base + 2])
                nc.vector.scalar_tensor_tensor(
                    SCRP[:], CCF[:], 1.0, OBJC[:], ALU.mult, ALU.mult,
                    accum_out=ACCD[:, base + 2:base + 3])

        # ---- sparse losses per layer (4 images batched on partitions) ----
        for li, lay in enumerate(LAYERS):
            W = lay["W"]
            Sp = SPA[li]
            obj = Sp[:, 0:1]
            WH1 = spap.tile([P, 1], F32, tag="wh1")
            nc.vector.tensor_mul(WH1[:], Sp[:, 3:4], Sp[:, 4:5])
            SC = spap.tile([P, 1], F32, tag="sc")
            nc.vector.tensor_scalar(SC[:], WH1[:], -1.0, 2.0, ALU.mult,
                                    ALU.add)
            OSC = spap.tile([P, 1], F32, tag="osc")
            nc.vector.tensor_mul(OSC[:], SC[:], obj)
            IV = spap.tile([P, 1], F32, tag="iv")
            nc.vector.tensor_scalar(IV[:], obj, -1.0, 1.0, ALU.mult, ALU.add)
            # xy
            EX = spap.tile([P, 2], F32, tag="ex")
            nc.scalar.activation(EX[:], Sp[:, 90:92], ACT.Exp, scale=-1.0)
            nc.vector.tensor_scalar_add(EX[:], EX[:], 1.0)
            SG = spap.tile([P, 2], F32, tag="sg")
            nc.vector.reciprocal(SG[:], EX[:])
            CX = spap.tile([P, 2], F32, tag="cx")
            nc.vector.tensor_add(CX[:], SG[:], Sp[:, 85:87])
            nc.vector.tensor_scalar_mul(CX[:], CX[:], 1.0 / W)
            TX = spap.tile([P, 2], F32, tag="tx")
            nc.vector.scalar_tensor_tensor(TX[:], Sp[:, 1:3], W, Sp[:, 85:87],
                                           ALU.mult, ALU.subtract)
            EB = spap.tile([P, 2], F32, tag="eb")
            nc.scalar.activation(EB[:], CX[:], ACT.Exp, scale=-1.0)
            LB = spap.tile([P, 2], F32, tag="lb")
            nc.scalar.activation(LB[:], EB[:], ACT.Ln, bias=1.0)
            OMT = spap.tile([P, 2], F32, tag="omt")
            nc.vector.tensor_scalar(OMT[:], TX[:], -1.0, 1.0, ALU.mult,
                                    ALU.add)
            VV = spap.tile([P, 2], F32, tag="vv")
            nc.vector.tensor_mul(VV[:], OMT[:], CX[:])
            nc.vector.tensor_add(VV[:], VV[:], LB[:])
            SCR2 = spap.tile([P, 2], F32, tag="scr2")
            nc.vector.tensor_scalar(SCR2[:], VV[:], OSC[:], 0.0, ALU.mult,
                                    ALU.add,
                                    accum_out=SACC[:, 3 * li:3 * li + 1])
            # wh
            T1 = spap.tile([P, 2], F32, tag="t1")
            nc.vector.tensor_scalar(T1[:], Sp[:, 3:5], IMG_W, IV[:], ALU.mult,
                                    ALU.add)
            nc.scalar.activation(T1[:], T1[:], ACT.Ln)
            T2 = spap.tile([P, 2], F32, tag="t2")
            nc.vector.tensor_scalar_add(T2[:], Sp[:, 87:89], IV[:])
            nc.scalar.activation(T2[:], T2[:], ACT.Ln)
            nc.vector.tensor_sub(T1[:], T1[:], T2[:])   # true_wh
            EW2 = spap.tile([P, 2], F32, tag="ew2")
            nc.scalar.activation(EW2[:], Sp[:, 92:94], ACT.Exp)
            AN = spap.tile([P, 2], F32, tag="an")
            nc.vector.tensor_scalar_mul(AN[:], Sp[:, 87:89], 1.0 / W)
            nc.vector.tensor_mul(EW2[:], EW2[:], AN[:])  # pred wh
            nc.vector.tensor_sub(T1[:], T1[:], EW2[:])
            DW2 = spap.tile([P, 2], F32, tag="dw2")
            nc.scalar.activation(DW2[:], T1[:], ACT.Square)
            OSC5 = spap.tile([P, 1], F32, tag="osc5")
            nc.vector.tensor_scalar_mul(OSC5[:], OSC[:], 0.5)
            nc.vector.tensor_scalar(SCR2[:], DW2[:], OSC5[:], 0.0, ALU.mult,
                                    ALU.add,
                                    accum_out=SACC[:, 3 * li + 1:3 * li + 2])
            # cls
            EC = spap.tile([P, 80], F32, tag="ec")
            nc.scalar.activation(EC[:], Sp[:, 94:174], ACT.Exp, scale=-1.0)
            nc.vector.tensor_scalar_add(EC[:], EC[:], 1.0)
            SGC = spap.tile([P, 80], F32, tag="sgc")
            nc.vector.reciprocal(SGC[:], EC[:])
            EB2 = spap.tile([P, 80], F32, tag="eb2")
            nc.scalar.activation(EB2[:], SGC[:], ACT.Exp, scale=-1.0)
            LB2 = spap.tile([P, 80], F32, tag="lb2")
            nc.scalar.activation(LB2[:], EB2[:], ACT.Ln, bias=1.0)
            OM2 = spap.tile([P, 80], F32, tag="om2")
            nc.vector.tensor_scalar(OM2[:], Sp[:, 5:85], -1.0, 1.0, ALU.mult,
                                    ALU.add)
            nc.vector.tensor_mul(OM2[:], OM2[:], SGC[:])
            nc.vector.tensor_add(OM2[:], OM2[:], LB2[:])
            SCR3 = spap.tile([P, 80], F32, tag="scr3")
            nc.vector.tensor_scalar(SCR3[:], OM2[:], obj, 0.0, ALU.mult,
                                    ALU.add,
                                    accum_out=SACC[:, 3 * li + 2:3 * li + 3])

        # ---- final combine ----
        AC3 = ACCD[:].rearrange("p (x t) -> p x t", t=3)
        TMP = accp.tile([P, B_CORE * 3], F32)
        nc.vector.tensor_add(TMP[:], AC3[:, :, 0], AC3[:, :, 1])
        nc.vector.tensor_tensor(TMP[:], TMP[:], AC3[:, :, 2], ALU.subtract)
        FIN = accp.tile([P, B_CORE], F32)
        nc.vector.tensor_reduce(
            FIN[:], TMP[:].rearrange("p (i l) -> p i l", l=3),
            axis=AX.X, op=ALU.add)
        FSP = accp.tile([P, 1], F32)
        nc.vector.tensor_reduce(FSP[:], SACC[:], axis=AX.X, op=ALU.add)
        PL = pso.tile([B_CORE, 1], F32, tag="pl")
        nc.tensor.matmul(PL[:], FIN[:], ON128[:], start=True, stop=False)
        nc.tensor.matmul(PL[:], SELC[:], FSP[:], start=False, stop=True)
        OUT = accp.tile([B_CORE, 1], F32)
        nc.scalar.copy(OUT[:], PL[:])
        nc.sync.dma_start(out=loss_d[:], in_=OUT[:])

    nc.finalize()
    return nc


def _prep_core_inputs(y_true, pred_13, pred_26, pred_52):
    consts, percell = _make_consts()
    yt85 = np.asarray(y_true).reshape(32, 10647, 85)
    yt = np.empty((32, 10647, YW), np.float16)
    yt[:, :, 0:85] = yt85
    yt[:, :, 85:89] = percell[None]
    ps32 = [np.asarray(p).reshape(32, -1, 85)
            for p in (pred_13, pred_26, pred_52)]
    ps = [np.ascontiguousarray(p.astype(np.float16)) for p in ps32]
    pf = np.ascontiguousarray(
        np.concatenate([p[:, :, 0:5] for p in ps32], axis=1))
    in_maps = []
    for c in range(N_CORES):
        sl = slice(c * B_CORE, (c + 1) * B_CORE)
        m = {"yt": yt[sl], "p0": ps[0][sl], "p1": ps[1][sl],
             "p2": ps[2][sl], "pf": pf[sl]}
        m.update(consts)
        in_maps.append(m)
    return in_maps


def kernel(y_true, pred_13, pred_26, pred_52):
    from concourse.bass_utils import run_bass_kernel_spmd

    Ms = [8, 16, 28]
    obj = np.asarray(y_true)[..., 0].reshape(32, 10647)
    cnt = [obj[:, LAYERS[i]["coff"]:LAYERS[i]["coff"] + LAYERS[i]["N"]]
           .sum(1).max() for i in range(3)]
    if any(cnt[i] > Ms[i] for i in range(3)):
        Ms = [64, 64, 64]
    key = tuple(Ms)
    if key not in _NC_CACHE:
        _NC_CACHE[key] = build_nc(Ms)
    nc = _NC_CACHE[key]

    in_maps = _prep_core_inputs(y_true, pred_13, pred_26, pred_52)
    res = run_bass_kernel_spmd(nc, in_maps, core_ids=list(range(N_CORES)))
    out = np.concatenate([r["loss"].reshape(B_CORE) for r in res.results])
    return out.astype(np.float32)



# revision 10
# speedup vs baseline: 3.6250x; 3.6250x over previous
"""YOLO loss (nms_detection) Trainium2 Bass kernel.

Data parallel over 8 NeuronCores (4 images per core). Host prep casts and
packs inputs; all loss math runs on device:

  - dense stream per core: 6 fp16 channels per cell (conf, obj, xy, wh
    logits) packed [img, 128, 84 slots, 6]; class/label channels never
    touch the device densely (they only matter at obj cells).
  - obj rows (labels+preds+grid aux) are host-gathered into a small fp32
    side tensor; the sparse xy/wh/cls losses are computed on device from
    those rows, batched over the 3 layers.
  - IoU ignore mask: decode boxes on device; per (img, layer) the
    [128, S, M, 2] min/max/sub ops keep the (x,y) pair as the packed
    innermost dim so DVE runs them in 2x mode; relu on ACT; the
    intersection product on Pool; threshold test 3*inter >= a1+a2.
  - images are permuted so each slot position gets similar box counts
    across cores (per-slot M is the max over its 8 images).
  - activation table usage is phased (sigmoid set, then exp/ln set) so
    only 2 LoadActFuncSets are emitted.
"""

from contextlib import ExitStack

import numpy as np

ANCHORS = np.array([[116., 90.], [156., 198.], [373., 326.],
                    [30., 61.], [62., 45.], [59., 119.],
                    [10., 13.], [16., 30.], [33., 23.]], dtype=np.float32)
IMG_W = 416.0
P = 128
B_CORE = 4
N_CORES = 8
NCH = 6            # dense channels: conf, obj, px, py, pw, ph
STOT = 84          # dense slots: 4 (l0) + 16 (l1) + 64 (l2)
CH = 180           # sparse row channels
MAXB = 64          # reference top_k cap on boxes per image per layer

# per-layer: N cells (pos*anchor), slots S, grid W, slot offset
LAYERS = [
    dict(N=507,  S=4,  W=13.0, goff=0),
    dict(N=2028, S=16, W=26.0, goff=4),
    dict(N=8112, S=64, W=52.0, goff=20),
]

_NC_CACHE = {}


def _make_consts():
    # gc16 [128, 84, 6] fp16: (1/W, 1/W, gx/W, gy/W, aw/2W, ah/2W)
    # gcv  [128, 84] f32: valid mask
    gc = np.zeros((P, STOT, NCH), np.float32)
    gcv = np.zeros((P, STOT), np.float32)
    for li, lay in enumerate(LAYERS):
        W, N, S, goff = lay["W"], lay["N"], lay["S"], lay["goff"]
        c = np.arange(P * S)
        pos = c // 3
        gx = (pos % W).astype(np.float32)
        gy = (pos // W).astype(np.float32)
        aw = ANCHORS[3 * li + (c % 3), 0]
        ah = ANCHORS[3 * li + (c % 3), 1]
        valid = (c < N).astype(np.float32)
        # cell c -> slot goff + c//128, partition c%128
        s = goff + c // P
        p = c % P
        gc[p, s, 0] = 1.0 / W
        gc[p, s, 1] = 1.0 / W
        gc[p, s, 2] = np.where(valid, gx / W, 0.0)
        gc[p, s, 3] = np.where(valid, gy / W, 0.0)
        gc[p, s, 4] = np.where(valid, aw / (2.0 * W), 0.0)
        gc[p, s, 5] = np.where(valid, ah / (2.0 * W), 0.0)
        gcv[p, s] = valid
    return gc.astype(np.float16), gcv


def _sel_mats(cap):
    # selection matrices for per-image sparse sums
    n_per = P // cap                     # images per sparse tile
    sels = []
    for h in range(B_CORE // n_per):     # one matrix per sparse tile
        m = np.zeros((P, B_CORE), np.float32)
        for g in range(n_per):
            img = h * n_per + g
            m[cap * g:cap * (g + 1), img] = 1.0
        sels.append(m)
    ones = np.ones((P, 1), np.float32)
    return sels, ones


def build_nc(Ms, cap):
    """Ms: [3][B_CORE] per-layer per-slot box counts. cap: 32 or 64."""
    import concourse.bass as bass
    import concourse.bacc as bacc
    import concourse.mybir as mybir
    from concourse.tile import TileContext

    F32 = mybir.dt.float32
    F16 = mybir.dt.float16
    ALU = mybir.AluOpType
    ACT = mybir.ActivationFunctionType
    AX = mybir.AxisListType

    n_per = P // cap                 # images per sparse tile
    n_sp = B_CORE // n_per           # number of sparse tiles
    btlen = sum(5 * Ms[l][j] for l in range(3) for j in range(B_CORE))
    btlen = max(btlen, 1)

    nc = bacc.Bacc()
    dn_d = nc.dram_tensor("dn", [B_CORE, P, STOT * NCH], F16,
                          kind="ExternalInput")
    sp_d = nc.dram_tensor("sp", [n_sp, P, 3 * CH], F32, kind="ExternalInput")
    bt_d = nc.dram_tensor("bt", [btlen], F16, kind="ExternalInput")
    gc_d = nc.dram_tensor("gc16", [P, STOT * NCH], F16, kind="ExternalInput")
    gv_d = nc.dram_tensor("gcv", [P, STOT], F32, kind="ExternalInput")
    se_d = nc.dram_tensor("sels", [P, n_sp * B_CORE + 1], F32,
                          kind="ExternalInput")
    loss_d = nc.dram_tensor("loss", [B_CORE, 1], F32, kind="ExternalOutput")

    def mkap(base, off_el, dims):
        return bass.AP(tensor=base.tensor, offset=base.offset + off_el,
                       ap=[base.ap[0]] + [list(d) for d in dims])

    with TileContext(nc) as tc, ExitStack() as ctx:
        cpool = ctx.enter_context(tc.tile_pool(name="consts", bufs=1))
        dpool = ctx.enter_context(tc.tile_pool(name="dense", bufs=1))
        ipool = ctx.enter_context(tc.tile_pool(name="iou", bufs=4))
        spool = ctx.enter_context(tc.tile_pool(name="sparse", bufs=1))
        pso = ctx.enter_context(
            tc.tile_pool(name="pso", bufs=1, space=bass.MemorySpace.PSUM))

        # ---- loads ----
        DN = dpool.tile([P, B_CORE, STOT, NCH], F16)
        dnf = DN[:]
        nc.sync.dma_start(
            out=mkap(dnf, 0, [[STOT * NCH, B_CORE], [1, STOT * NCH]]),
            in_=bass.AP(tensor=dn_d[:].tensor, offset=0,
                        ap=[[STOT * NCH, P], [P * STOT * NCH, B_CORE],
                            [1, STOT * NCH]]))
        GC = cpool.tile([P, STOT, NCH], F16)
        nc.sync.dma_start(out=GC[:], in_=gc_d[:])
        GV = cpool.tile([P, STOT], F32)
        nc.sync.dma_start(out=GV[:], in_=gv_d[:])
        SEL = cpool.tile([P, n_sp * B_CORE + 1], F32)
        nc.sync.dma_start(out=SEL[:], in_=se_d[:])
        SPT = [spool.tile([P, 3, CH], F32, name=f"spt{h}")
               for h in range(n_sp)]
        for h in range(n_sp):
            nc.sync.dma_start(
                out=mkap(SPT[h][:], 0, [[1, 3 * CH]]),
                in_=bass.AP(tensor=sp_d[:].tensor, offset=h * P * 3 * CH,
                            ap=[[3 * CH, P], [1, 3 * CH]]))
        BT = cpool.tile([P, btlen], F16)
        nc.sync.dma_start(
            out=BT[:],
            in_=bass.AP(tensor=bt_d[:].tensor, offset=0,
                        ap=[[0, P], [1, btlen]]))

        btf = BT[:]
        gcf = GC[:]

        def img4(off_el, dims):
            # const view broadcast over the 4-image dim
            return bass.AP(tensor=gcf.tensor, offset=gcf.offset + off_el,
                           ap=[gcf.ap[0], [0, B_CORE]] + [list(d) for d in dims])

        # ================= ACT phase 1: sigmoid set =================
        SXY = dpool.tile([P, B_CORE, STOT, 2], F16)
        nc.scalar.activation(SXY[:], DN[:, :, :, 2:4], ACT.Sigmoid)
        C = dpool.tile([P, B_CORE, STOT], F32)
        nc.scalar.activation(C[:], DN[:, :, :, 0], ACT.Sigmoid)
        SPS = [spool.tile([P, 3, 2], F32, name=f"sps{h}") for h in range(n_sp)]
        SPCg = [spool.tile([P, 3, 80], F32, name=f"spc{h}")
                for h in range(n_sp)]
        for h in range(n_sp):
            nc.scalar.activation(SPS[h][:], SPT[h][:, :, 16:18], ACT.Sigmoid)
            nc.scalar.activation(SPCg[h][:], SPT[h][:, :, 100:180],
                                 ACT.Sigmoid)

        # ================= dense decode (DVE) =================
        CXY = dpool.tile([P, B_CORE, STOT, 2], F16)
        nc.vector.tensor_tensor(CXY[:], SXY[:],
                                img4(0, [[NCH, STOT], [1, 2]]), ALU.mult)
        nc.vector.tensor_tensor(CXY[:], CXY[:],
                                img4(2, [[NCH, STOT], [1, 2]]), ALU.add)

        # sparse logit chains (DVE) so all Exps can precede all Lns
        CXs, TXY, ECX, EPW, EC2 = {}, {}, {}, {}, {}
        LCX, LC2, TWH = {}, {}, {}
        for h in range(n_sp):
            CXs[h] = spool.tile([P, 3, 2], F32, name=f"cxs{h}")
            nc.vector.tensor_tensor(
                CXs[h][:], SPS[h][:],
                SPT[h][:, :, 9:10].broadcast_to([P, 3, 2]), ALU.mult)
            nc.vector.tensor_add(CXs[h][:], CXs[h][:], SPT[h][:, :, 7:9])
            TXY[h] = spool.tile([P, 3, 2], F32, name=f"txy{h}")
            nc.vector.tensor_tensor(
                TXY[h][:], SPT[h][:, :, 1:3],
                SPT[h][:, :, 10:11].broadcast_to([P, 3, 2]), ALU.mult)
            nc.vector.tensor_sub(TXY[h][:], TXY[h][:], SPT[h][:, :, 5:7])

        # ================= ACT phase 2: all Exp, then all Ln =================
        EWH = dpool.tile([P, B_CORE, STOT, 2], F16)
        nc.scalar.activation(EWH[:], DN[:, :, :, 4:6], ACT.Exp)
        E2 = dpool.tile([P, B_CORE, STOT], F32)
        nc.scalar.activation(E2[:], C[:], ACT.Exp, scale=-1.0)
        for h in range(n_sp):
            ECX[h] = spool.tile([P, 3, 2], F32, name=f"ecx{h}")
            nc.scalar.activation(ECX[h][:], CXs[h][:], ACT.Exp, scale=-1.0)
            EPW[h] = spool.tile([P, 3, 2], F32, name=f"epw{h}")
            nc.scalar.activation(EPW[h][:], SPT[h][:, :, 18:20], ACT.Exp)
            EC2[h] = spool.tile([P, 3, 80], F32, name=f"ec2{h}")
            nc.scalar.activation(EC2[h][:], SPCg[h][:], ACT.Exp, scale=-1.0)
        L1 = dpool.tile([P, B_CORE, STOT], F32)
        nc.scalar.activation(L1[:], E2[:], ACT.Ln, bias=1.0)
        for h in range(n_sp):
            LCX[h] = spool.tile([P, 3, 2], F32, name=f"lcx{h}")
            nc.scalar.activation(LCX[h][:], ECX[h][:], ACT.Ln, bias=1.0)
            LC2[h] = spool.tile([P, 3, 80], F32, name=f"lc2{h}")
            nc.scalar.activation(LC2[h][:], EC2[h][:], ACT.Ln, bias=1.0)
            TWH[h] = spool.tile([P, 3, 2], F32, name=f"twh{h}")
            nc.scalar.activation(TWH[h][:], SPT[h][:, :, 3:5], ACT.Ln)

        HWT = dpool.tile([P, B_CORE, STOT, 2], F16)
        nc.vector.tensor_tensor(HWT[:], EWH[:],
                                img4(4, [[NCH, STOT], [1, 2]]), ALU.mult)
        PMX = dpool.tile([P, B_CORE, STOT, 2], F16)
        nc.vector.tensor_add(PMX[:], CXY[:], HWT[:])
        PMN = dpool.tile([P, B_CORE, STOT, 2], F16)
        nc.vector.tensor_sub(PMN[:], CXY[:], HWT[:])
        A13 = dpool.tile([P, B_CORE, STOT], F32)
        nc.vector.scalar_tensor_tensor(A13[:], HWT[:, :, :, 0], 4.0 / 3.0,
                                       HWT[:, :, :, 1], ALU.mult, ALU.mult)

        SMX = dpool.tile([P, B_CORE, STOT], F32)
        nc.vector.memset(SMX[:], -1.0e4)

        # ================= IoU per (img, layer) =================
        pmxf, pmnf = PMX[:], PMN[:]
        bto = 0
        bt_off = {}
        for li in range(3):
            for j in range(B_CORE):
                m = Ms[li][j]
                bt_off[(li, j)] = bto
                bto += 5 * m
        for j in range(B_CORE):
            for li in (2, 1, 0):
                M = Ms[li][j]
                if M == 0:
                    continue
                lay = LAYERS[li]
                S, goff = lay["S"], lay["goff"]
                o = bt_off[(li, j)]
                pmxv = mkap(pmxf, (j * STOT + goff) * 2,
                            [[2, S], [0, M], [1, 2]])
                pmnv = mkap(pmnf, (j * STOT + goff) * 2,
                            [[2, S], [0, M], [1, 2]])
                btpv = mkap(btf, o, [[0, S], [2, M], [1, 2]])
                btnv = mkap(btf, o + 2 * M, [[0, S], [2, M], [1, 2]])
                bt4v = mkap(btf, o + 4 * M, [[0, S], [1, M]])

                I = ipool.tile([P, S, M, 2], F16, tag=f"i{li}",
                               name=f"i{li}_{j}")
                nc.vector.tensor_tensor(I[:], pmxv, btpv, ALU.min)
                J = ipool.tile([P, S, M, 2], F16, tag=f"j{li}",
                               name=f"j{li}_{j}")
                nc.vector.tensor_tensor(J[:], pmnv, btnv, ALU.max)
                WD = ipool.tile([P, S, M, 2], F16, tag=f"w{li}",
                                name=f"w{li}_{j}")
                nc.vector.tensor_sub(WD[:], I[:], J[:])
                wdf = WD[:]
                WRX = ipool.tile([P, S, M], F16, tag=f"r{li}",
                                 name=f"r{li}_{j}")
                nc.scalar.activation(WRX[:], mkap(wdf, 0, [[2 * M, S], [2, M]]),
                                     ACT.Relu)
                WDY = ipool.tile([P, S, M], F16, tag=f"y{li}",
                                 name=f"y{li}_{j}")
                nc.scalar.copy(WDY[:], mkap(wdf, 1, [[2 * M, S], [2, M]]))
                PRD = ipool.tile([P, S, M], F16, tag=f"p{li}",
                                 name=f"p{li}_{j}")
                nc.vector.tensor_mul(PRD[:], WRX[:], WDY[:])
                T = ipool.tile([P, S, M], F16, tag=f"t{li}",
                               name=f"t{li}_{j}")
                nc.vector.tensor_tensor(T[:], PRD[:], bt4v, ALU.subtract)
                smxv = mkap(SMX[:], j * STOT + goff, [[1, S]])
                nc.vector.tensor_reduce(smxv, T[:], axis=AX.X, op=ALU.max)

        # ================= dense conf loss =================
        OBJ = dpool.tile([P, B_CORE, STOT], F32)
        nc.vector.tensor_copy(OBJ[:], DN[:, :, :, 1])
        CMP = dpool.tile([P, B_CORE, STOT], F32)
        nc.vector.tensor_tensor(CMP[:], SMX[:], A13[:], ALU.is_lt)
        gvv = bass.AP(tensor=GV[:].tensor, offset=GV[:].offset,
                      ap=[GV[:].ap[0], [0, B_CORE], [1, STOT]])
        nc.vector.tensor_tensor(CMP[:], CMP[:], gvv, ALU.mult)
        WT = dpool.tile([P, B_CORE, STOT], F32)
        nc.vector.tensor_tensor(WT[:], CMP[:], OBJ[:], ALU.max)
        F = dpool.tile([P, B_CORE, STOT], F32)
        nc.vector.tensor_add(F[:], C[:], L1[:])
        nc.vector.tensor_mul(F[:], F[:], WT[:])
        R1 = dpool.tile([P, B_CORE], F32)
        nc.vector.tensor_reduce(R1[:], F[:], axis=AX.X, op=ALU.add)
        S2 = dpool.tile([P, B_CORE, STOT], F32)
        nc.vector.tensor_mul(S2[:], C[:], OBJ[:])
        R2 = dpool.tile([P, B_CORE], F32)
        nc.vector.tensor_reduce(R2[:], S2[:], axis=AX.X, op=ALU.add)
        FIN = dpool.tile([P, B_CORE], F32)
        nc.vector.tensor_sub(FIN[:], R1[:], R2[:])

        # ================= sparse losses =================
        SACC = spool.tile([P, n_sp, 3], F32)
        for h in range(n_sp):
            Sp = SPT[h][:]
            obj = SPT[h][:, :, 0:1]

            WH1 = spool.tile([P, 3], F32, name=f"wh1{h}")
            nc.vector.tensor_mul(WH1[:], SPT[h][:, :, 3], SPT[h][:, :, 4])
            SC = spool.tile([P, 3], F32, name=f"sc{h}")
            nc.vector.tensor_scalar(SC[:], WH1[:], -1.0, 2.0, ALU.mult,
                                    ALU.add)
            OSC = spool.tile([P, 3], F32, name=f"osc{h}")
            nc.vector.tensor_mul(OSC[:], SC[:], SPT[h][:, :, 0])
            oscv = OSC[:].broadcast_to([P, 3, 2])

            # xy bce on logits cx in (0,1)
            OMT = spool.tile([P, 3, 2], F32, name=f"omt{h}")
            nc.vector.tensor_scalar(OMT[:], TXY[h][:], -1.0, 1.0, ALU.mult,
                                    ALU.add)
            nc.vector.tensor_mul(OMT[:], OMT[:], CXs[h][:])
            nc.vector.tensor_add(OMT[:], OMT[:], LCX[h][:])
            SCR = spool.tile([P, 3, 2], F32, name=f"scr{h}")
            nc.vector.scalar_tensor_tensor(
                SCR[:], OMT[:], 1.0, oscv, ALU.mult, ALU.mult,
                accum_out=mkap(SACC[:], h * 3 + 0, [[1, 1]]))

            # wh squared error
            nc.vector.tensor_add(TWH[h][:], TWH[h][:], SPT[h][:, :, 13:15])
            nc.vector.tensor_mul(EPW[h][:], EPW[h][:], SPT[h][:, :, 11:13])
            nc.vector.tensor_sub(TWH[h][:], TWH[h][:], EPW[h][:])
            SQD = spool.tile([P, 3, 2], F32, name=f"sqd{h}")
            nc.scalar.activation(SQD[:], TWH[h][:], ACT.Square)
            SCR2 = spool.tile([P, 3, 2], F32, name=f"scr2{h}")
            nc.vector.scalar_tensor_tensor(
                SCR2[:], SQD[:], 0.5, oscv, ALU.mult, ALU.mult,
                accum_out=mkap(SACC[:], h * 3 + 1, [[1, 1]]))

            # cls bce on logits cls in (0,1)
            OM2 = spool.tile([P, 3, 80], F32, name=f"om2{h}")
            nc.vector.tensor_scalar(OM2[:], SPT[h][:, :, 20:100], -1.0, 1.0,
                                    ALU.mult, ALU.add)
            nc.vector.tensor_mul(OM2[:], OM2[:], SPCg[h][:])
            nc.vector.tensor_add(OM2[:], OM2[:], LC2[h][:])
            SCR3 = spool.tile([P, 3, 80], F32, name=f"scr3{h}")
            nc.vector.scalar_tensor_tensor(
                SCR3[:], OM2[:], 1.0, obj.broadcast_to([P, 3, 80]),
                ALU.mult, ALU.mult,
                accum_out=mkap(SACC[:], h * 3 + 2, [[1, 1]]))

        SSUM = spool.tile([P, n_sp], F32)
        nc.vector.tensor_reduce(SSUM[:], SACC[:], axis=AX.X, op=ALU.add)

        # ================= final combine =================
        PL = pso.tile([B_CORE, 1], F32)
        nc.tensor.matmul(PL[:], FIN[:], SEL[:, n_sp * B_CORE:], start=True,
                         stop=False)
        for h in range(n_sp):
            nc.tensor.matmul(PL[:], SEL[:, h * B_CORE:(h + 1) * B_CORE],
                             SSUM[:, h:h + 1], start=False, stop=(h == n_sp - 1))
        OUT = spool.tile([B_CORE, 1], F32)
        nc.scalar.copy(OUT[:], PL[:])
        nc.sync.dma_start(out=loss_d[:], in_=OUT[:])

    nc.finalize()
    return nc


def _plan(y_true):
    """Box counts -> image permutation + per-slot Ms + cap."""
    yt = np.asarray(y_true).reshape(32, 10647, 85)
    offs = [0, 507, 2535, 10647]
    counts = np.zeros((32, 3), np.int64)
    for li in range(3):
        counts[:, li] = (yt[:, offs[li]:offs[li + 1], 0] > 0.5).sum(1)
    order = np.argsort(counts[:, 2], kind="stable")
    perm = order.reshape(B_CORE, N_CORES)       # [slot, core] -> image
    cnt_cap = np.minimum(counts, MAXB)          # boxes capped like top_k
    Ms = [[int(cnt_cap[perm[j], li].max()) for j in range(B_CORE)]
          for li in range(3)]
    cap = 32 if counts.max() <= 32 else 64
    if counts.max() > MAXB:
        raise NotImplementedError("more than 64 true boxes per image/layer")
    return perm, Ms, cap, counts


def _prep_core_inputs(y_true, pred_13, pred_26, pred_52):
    yt = np.asarray(y_true).reshape(32, 10647, 85).astype(np.float32)
    ps = np.concatenate([np.asarray(p).reshape(32, -1, 85)
                         for p in (pred_13, pred_26, pred_52)],
                        axis=1).astype(np.float32)
    perm, Ms, cap, counts = _plan(y_true)
    n_per = P // cap
    n_sp = B_CORE // n_per

    offs = [0, 507, 2535]
    # dense packed stream [32, 128, 84, 6] fp16
    dn = np.zeros((32, P, STOT, NCH), np.float16)
    for li, lay in enumerate(LAYERS):
        N, S, goff = lay["N"], lay["S"], lay["goff"]
        c = np.arange(N)
        s = goff + c // P
        p = c % P
        cell = offs[li] + c
        dn[:, p, s, 0] = ps[:, cell, 0]
        dn[:, p, s, 1] = yt[:, cell, 0]
        dn[:, p, s, 2:6] = ps[:, cell, 1:5]

    # per-layer aux (grid / anchors) per flat cell
    aux = np.zeros((10647, 10), np.float32)
    for li, lay in enumerate(LAYERS):
        W, N = lay["W"], lay["N"]
        c = np.arange(N)
        pos = c // 3
        gx = (pos % W).astype(np.float32)
        gy = (pos // W).astype(np.float32)
        aw = ANCHORS[3 * li + (c % 3), 0]
        ah = ANCHORS[3 * li + (c % 3), 1]
        r = slice(offs[li], offs[li] + N)
        aux[r, 0] = gx
        aux[r, 1] = gy
        aux[r, 2] = gx / W
        aux[r, 3] = gy / W
        aux[r, 4] = 1.0 / W
        aux[r, 5] = W
        aux[r, 6] = aw / W
        aux[r, 7] = ah / W
        aux[r, 8] = np.log(IMG_W / aw)
        aux[r, 9] = np.log(IMG_W / ah)

    gc16, gcv = _make_consts()
    sels, ones = _sel_mats(cap)
    selcat = np.concatenate(sels + [ones], axis=1)

    in_maps = []
    for core in range(N_CORES):
        imgs = [int(perm[j, core]) for j in range(B_CORE)]
        dnc = np.ascontiguousarray(dn[imgs])

        sp = np.zeros((n_sp, P, 3, CH), np.float32)
        sp[:, :, :, 3:5] = 1.0     # pad yt wh -> ln() finite
        sp[:, :, :, 6 + 2] = 1.0   # pad invW etc: any finite nonzero
        sp[:, :, :, 11:13] = 1.0
        bt_parts = []
        for li in range(3):
            base = offs[li]
            N = LAYERS[li]["N"]
            for j in range(B_CORE):
                img = imgs[j]
                M = Ms[li][j]
                idx = np.nonzero(yt[img, base:base + N, 0] > 0.5)[0][:MAXB]
                k = len(idx)
                cells = base + idx
                if M > 0:
                    btp = np.full((M, 2), -100.0, np.float32)
                    btn = np.full((M, 2), 100.0, np.float32)
                    bt4 = np.full((M,), 100.0, np.float32)
                    if k:
                        bxy = yt[img, cells, 1:3]
                        bwh = yt[img, cells, 3:5]
                        btp[:k] = bxy + bwh * 0.5
                        btn[:k] = bxy - bwh * 0.5
                        bt4[:k] = bwh[:, 0] * bwh[:, 1] / 3.0
                    bt_parts += [btp.ravel(), btn.ravel(), bt4]
                if k:
                    h, g = j // n_per, j % n_per
                    rows = slice(cap * g, cap * g + k)
                    sp[h, rows, li, 0] = 1.0
                    sp[h, rows, li, 1:5] = yt[img, cells, 1:5]
                    sp[h, rows, li, 5:15] = aux[cells]
                    sp[h, rows, li, 16:20] = ps[img, cells, 1:5]
                    sp[h, rows, li, 20:100] = yt[img, cells, 5:85]
                    sp[h, rows, li, 100:180] = ps[img, cells, 5:85]
        bt = (np.concatenate(bt_parts).astype(np.float16) if bt_parts
              else np.zeros((1,), np.float16))
        m = {"dn": dnc.reshape(B_CORE, P, STOT * NCH),
             "sp": sp.reshape(n_sp, P, 3 * CH),
             "bt": bt,
             "gc16": gc16.reshape(P, STOT * NCH),
             "gcv": gcv,
             "sels": selcat}
        in_maps.append(m)
    return in_maps


def kernel(y_true, pred_13, pred_26, pred_52):
    from concourse.bass_utils import run_bass_kernel_spmd

    perm, Ms, cap, counts = _plan(y_true)
    key = (tuple(tuple(m) for m in Ms), cap)
    if key not in _NC_CACHE:
        _NC_CACHE[key] = build_nc(Ms, cap)
    nc = _NC_CACHE[key]

    in_maps = _prep_core_inputs(y_true, pred_13, pred_26, pred_52)
    res = run_bass_kernel_spmd(nc, in_maps, core_ids=list(range(N_CORES)))
    out = np.zeros((32,), np.float32)
    for core in range(N_CORES):
        vals = res.results[core]["loss"].reshape(B_CORE)
        for j in range(B_CORE):
            out[perm[j, core]] = vals[j]
    return out
